# revision 12
# baseline (speedup 1.0000x reference)
import sys

sys.path.insert(0, "/opt/trn_rl_repo")

import numpy as np

B, N, D = 4, 4096, 1024
E, H = 8, 2752
HP = 2816
T = B * N
NCORES = 8
TC = T // NCORES
CAP = 640
P = 128
TT = TC // P
DC = D // P
HC = HP // P
CT = CAP // P
TKC = 320
W13_CHUNK = 256
W2_HALF = HC // 2


def _build(phases=("router", "expert", "combine"), use_silu=True):
    import concourse.bass as bass
    import concourse.bacc as bacc
    import concourse.mybir as mybir
    import concourse.tile as tile
    import concourse.tile_utils as tile_utils
    from concourse.masks import make_identity

    tile_utils.max_sbuf_usage = 204 * 1024

    f32 = mybir.dt.float32
    f32r = mybir.dt.float32r
    i32 = mybir.dt.int32
    AF = mybir.ActivationFunctionType

    nc = bacc.Bacc(None, target_bir_lowering=False)

    n13 = HP // W13_CHUNK
    x_in = nc.declare_dram_parameter("x", [TC, D], f32, isOutput=False)
    rwt_in = nc.declare_dram_parameter("rwt", [D, E], f32, isOutput=False)
    tri_in = nc.declare_dram_parameter("tri", [P, P], f32, isOutput=False)
    w1_in = nc.declare_dram_parameter(
        "w1", [E, n13, P, DC, W13_CHUNK], f32r, isOutput=False
    )
    w3_in = nc.declare_dram_parameter(
        "w3", [E, n13, P, DC, W13_CHUNK], f32r, isOutput=False
    )
    w2_in = nc.declare_dram_parameter(
        "w2", [E, DC, 2, P, W2_HALF, P], f32r, isOutput=False
    )
    out_p = nc.declare_dram_parameter("out", [TC, D], f32, isOutput=True)

    with tile.TileContext(nc) as tc:
        with (
            tc.tile_pool(name="const", bufs=1) as const,
            tc.tile_pool(name="xs", bufs=2) as xs_pool,
            tc.tile_pool(name="sm", bufs=2) as sm_pool,
            tc.tile_pool(name="silp", bufs=2) as sil_pool,
            tc.tile_pool(name="idx", bufs=TT + 1) as idx_pool,
            tc.tile_pool(name="xet", bufs=1) as xet_pool,
            tc.tile_pool(name="wst", bufs=2) as wst_pool,
            tc.tile_pool(name="hht", bufs=HC + 1) as hht_pool,
            tc.tile_pool(name="yts", bufs=DC + 1) as yts_pool,
            tc.tile_pool(name="yo", bufs=2) as yo_pool,
            tc.tile_pool(name="dram", bufs=1, space="DRAM") as dram,
            tc.tile_pool(name="psm", bufs=2, space="PSUM") as psm,
            tc.tile_pool(name="pbig", bufs=3, space="PSUM") as pbig,
        ):
            XE = dram.tile([E * CAP, D], f32)
            Y = dram.tile([E * CAP, D], f32)

            ident = const.tile([P, P], f32)
            make_identity(nc, ident[:])
            tri = const.tile([P, P], f32)
            nc.sync.dma_start(out=tri[:], in_=tri_in[:])
            rwts = const.tile([P, DC, E], f32)
            nc.sync.dma_start(
                out=rwts[:], in_=rwt_in.rearrange("(a p) e -> p a e", p=P)
            )
            ones_row = const.tile([1, P], f32)
            nc.vector.memset(ones_row[:], 1.0)
            ones_col = const.tile([P, 1], f32)
            nc.vector.memset(ones_col[:], 1.0)
            ecap = const.tile([P, E], f32)
            for e in range(E):
                nc.vector.memset(ecap[:, e : e + 1], float(e * CAP))
            runoff = const.tile([1, E], f32)
            nc.vector.memset(runoff[:], 0.0)

            zt = const.tile([P, D], f32)
            nc.vector.memset(zt[:], 0.0)
            for w in range(E * CT):
                nc.sync.dma_start(out=XE[:][w * P : (w + 1) * P, :], in_=zt[:])

            flat0s, flat1s, g0s, g1s = [], [], [], []

            for i in range(TT):
                xt = xs_pool.tile([P, D], f32, name="xst")
                nc.sync.dma_start(out=xt[:], in_=x_in[i * P : (i + 1) * P, :])

                plog = psm.tile([P, E], f32, space="PSUM", name="ptr")
                for dc in range(DC):
                    ptr = psm.tile([P, P], f32, space="PSUM", name="ptr")
                    nc.tensor.transpose(
                        out=ptr[:], in_=xt[:, dc * P : (dc + 1) * P], identity=ident[:]
                    )
                    xtb = sm_pool.tile([P, P], f32, name="xtb")
                    nc.vector.tensor_copy(out=xtb[:], in_=ptr[:])
                    nc.tensor.matmul(
                        out=plog[:],
                        lhsT=xtb[:],
                        rhs=rwts[:, dc, :],
                        start=(dc == 0),
                        stop=(dc == DC - 1),
                    )
                lg = sm_pool.tile([P, E], f32, name="lg")
                nc.vector.tensor_copy(out=lg[:], in_=plog[:])

                top8 = sm_pool.tile([P, 8], f32, name="top8")
                nc.vector.max(out=top8[:], in_=lg[:])
                m1 = top8[:, 0:1]
                m2 = top8[:, 1:2]
                mask0 = sm_pool.tile([P, E], f32, name="mask0")
                nc.vector.tensor_tensor(
                    out=mask0[:],
                    in0=lg[:],
                    in1=m1.to_broadcast([P, E]),
                    op=mybir.AluOpType.is_equal,
                )
                mask1 = sm_pool.tile([P, E], f32, name="mask1")
                nc.vector.tensor_tensor(
                    out=mask1[:],
                    in0=lg[:],
                    in1=m2.to_broadcast([P, E]),
                    op=mybir.AluOpType.is_equal,
                )

                dneg = sm_pool.tile([P, 1], f32, name="dneg")
                nc.vector.tensor_sub(out=dneg[:], in0=m2, in1=m1)
                e2 = sm_pool.tile([P, 1], f32, name="e2")
                nc.scalar.activation(e2[:], dneg[:], AF.Exp)
                den = sm_pool.tile([P, 1], f32, name="den")
                nc.vector.tensor_scalar_add(den[:], e2[:], 1.0)
                g0 = idx_pool.tile([P, 1], f32, name="g0")
                nc.vector.reciprocal(g0[:], den[:])
                g1 = idx_pool.tile([P, 1], f32, name="g1")
                nc.vector.tensor_sub(out=g1[:], in0=ones_col[:], in1=g0[:])
                g0s.append(g0)
                g1s.append(g1)

                m01 = sm_pool.tile([P, E], f32, name="m01")
                nc.vector.tensor_add(out=m01[:], in0=mask0[:], in1=mask1[:])
                ppos = psm.tile([P, E], f32, space="PSUM", name="ptr")
                nc.tensor.matmul(
                    out=ppos[:], lhsT=tri[:], rhs=m01[:], start=True, stop=False
                )
                nc.tensor.matmul(
                    out=ppos[:], lhsT=ones_row[:], rhs=runoff[:], start=False, stop=True
                )
                pbs = psm.tile([1, E], f32, space="PSUM", name="ptr")
                nc.tensor.matmul(
                    out=pbs[:], lhsT=ones_col[:], rhs=m01[:], start=True, stop=True
                )
                pos = sm_pool.tile([P, E], f32, name="pos")
                nc.vector.tensor_copy(out=pos[:], in_=ppos[:])
                nc.vector.tensor_add(out=runoff[:], in0=runoff[:], in1=pbs[:])
                nc.vector.tensor_scalar_min(pos[:], pos[:], float(CAP - 1))
                nc.vector.tensor_add(out=pos[:], in0=pos[:], in1=ecap[:])

                tmp = sm_pool.tile([P, E], f32, name="tmpf")
                flatf = sm_pool.tile([P, 1], f32, name="flatf")
                flat0 = idx_pool.tile([P, 1], i32, name="flat0")
                nc.vector.tensor_mul(out=tmp[:], in0=pos[:], in1=mask0[:])
                nc.vector.reduce_sum(out=flatf[:], in_=tmp[:], axis=mybir.AxisListType.X)
                nc.vector.tensor_copy(out=flat0[:], in_=flatf[:])
                flat1 = idx_pool.tile([P, 1], i32, name="flat1")
                nc.vector.tensor_mul(out=tmp[:], in0=pos[:], in1=mask1[:])
                nc.vector.reduce_sum(out=flatf[:], in_=tmp[:], axis=mybir.AxisListType.X)
                nc.vector.tensor_copy(out=flat1[:], in_=flatf[:])
                flat0s.append(flat0)
                flat1s.append(flat1)

                nc.gpsimd.indirect_dma_start(
                    out=XE[:][:],
                    out_offset=bass.IndirectOffsetOnAxis(ap=flat0[:, :1], axis=0),
                    in_=xt[:],
                    in_offset=None,
                )
                nc.gpsimd.indirect_dma_start(
                    out=XE[:][:],
                    out_offset=bass.IndirectOffsetOnAxis(ap=flat1[:, :1], axis=0),
                    in_=xt[:],
                    in_offset=None,
                )

            for e in range(E):
                xet = [
                    xet_pool.tile([P, CAP], f32r, name=f"xet{dc}") for dc in range(DC)
                ]
                for j in range(CT):
                    xl = xs_pool.tile([P, D], f32, name="xst")
                    nc.sync.dma_start(
                        out=xl[:], in_=XE[:][e * CAP + j * P : e * CAP + (j + 1) * P, :]
                    )
                    for dc in range(DC):
                        ptr = psm.tile([P, P], f32, space="PSUM", name="ptr")
                        nc.tensor.transpose(
                            out=ptr[:],
                            in_=xl[:, dc * P : (dc + 1) * P],
                            identity=ident[:],
                        )
                        nc.vector.tensor_copy(
                            out=xet[dc][:, j * P : (j + 1) * P], in_=ptr[:]
                        )

                hht = []
                for ci in range(n13):
                    w1s = wst_pool.tile([P, DC, W13_CHUNK], f32r, name="w1s")
                    nc.sync.dma_start(out=w1s[:], in_=w1_in[e, ci])
                    w3s = wst_pool.tile([P, DC, W13_CHUNK], f32r, name="w3s")
                    nc.sync.dma_start(out=w3s[:], in_=w3_in[e, ci])
                    for hh_i in range(W13_CHUNK // P):
                        p1 = pbig.tile([P, 2, 512], f32, space="PSUM", name="pbig")
                        p3 = pbig.tile([P, 2, 512], f32, space="PSUM", name="pbig")
                        hsl = slice(hh_i * P, (hh_i + 1) * P)
                        for dc in range(DC):
                            for ck in range(2):
                                tsl = slice(ck * TKC, (ck + 1) * TKC)
                                nc.tensor.matmul(
                                    out=p1[:, ck, :TKC],
                                    lhsT=w1s[:, dc, hsl],
                                    rhs=xet[dc][:, tsl],
                                    start=(dc == 0),
                                    stop=(dc == DC - 1),
                                )
                                nc.tensor.matmul(
                                    out=p3[:, ck, :TKC],
                                    lhsT=w3s[:, dc, hsl],
                                    rhs=xet[dc][:, tsl],
                                    start=(dc == 0),
                                    stop=(dc == DC - 1),
                                )
                        sil = sil_pool.tile([P, CAP], f32, name="sil")
                        if use_silu:
                            nc.scalar.activation(sil[:], p1[:, :, :TKC], AF.Silu)
                        else:
                            nc.scalar.activation(sil[:], p1[:, :, :TKC], AF.Sigmoid)
                            nc.vector.tensor_mul(
                                out=sil[:], in0=sil[:], in1=p1[:, :, :TKC]
                            )
                        ht = hht_pool.tile([P, CAP], f32r, name="hht")
                        nc.vector.tensor_mul(
                            out=ht[:], in0=sil[:], in1=p3[:, :, :TKC]
                        )
                        hht.append(ht)

                yts = []
                for dc in range(DC):
                    py = pbig.tile([P, 2, 512], f32, space="PSUM", name="pbig")
                    for half in range(2):
                        w2s = wst_pool.tile([P, W2_HALF, P], f32r, name="w2s")
                        nc.sync.dma_start(out=w2s[:], in_=w2_in[e, dc, half])
                        for hi in range(W2_HALF):
                            hc = half * W2_HALF + hi
                            for ck in range(2):
                                tsl = slice(ck * TKC, (ck + 1) * TKC)
                                nc.tensor.matmul(
                                    out=py[:, ck, :TKC],
                                    lhsT=w2s[:, hi, :],
                                    rhs=hht[hc][:, tsl],
                                    start=(hc == 0),
                                    stop=(hc == HC - 1),
                                )
                    yt = yts_pool.tile([P, CAP], f32, name="yts")
                    nc.scalar.activation(yt[:], py[:, :, :TKC], AF.Copy)
                    yts.append(yt)

                for j in range(CT):
                    yo = yo_pool.tile([P, D], f32, name="yo")
                    for dc in range(DC):
                        ptr = psm.tile([P, P], f32, space="PSUM", name="ptr")
                        nc.tensor.transpose(
                            out=ptr[:],
                            in_=yts[dc][:, j * P : (j + 1) * P],
                            identity=ident[:],
                        )
                        nc.vector.tensor_copy(
                            out=yo[:, dc * P : (dc + 1) * P], in_=ptr[:]
                        )
                    nc.sync.dma_start(
                        out=Y[:][e * CAP + j * P : e * CAP + (j + 1) * P, :],
                        in_=yo[:],
                    )

            for i in range(TT):
                ga = xs_pool.tile([P, D], f32, name="ga")
                nc.gpsimd.indirect_dma_start(
                    out=ga[:],
                    out_offset=None,
                    in_=Y[:][:],
                    in_offset=bass.IndirectOffsetOnAxis(ap=flat0s[i][:, :1], axis=0),
                )
                gb = xs_pool.tile([P, D], f32, name="gb")
                nc.gpsimd.indirect_dma_start(
                    out=gb[:],
                    out_offset=None,
                    in_=Y[:][:],
                    in_offset=bass.IndirectOffsetOnAxis(ap=flat1s[i][:, :1], axis=0),
                )
                nc.vector.tensor_mul(
                    out=ga[:], in0=ga[:], in1=g0s[i][:].to_broadcast([P, D])
                )
                nc.vector.tensor_mul(
                    out=gb[:], in0=gb[:], in1=g1s[i][:].to_broadcast([P, D])
                )
                nc.vector.tensor_add(out=ga[:], in0=ga[:], in1=gb[:])
                nc.sync.dma_start(out=out_p[i * P : (i + 1) * P, :], in_=ga[:])

    nc.compile()
    return nc


_NC_CACHE = None


def _get_nc():
    global _NC_CACHE
    if _NC_CACHE is None:
        _NC_CACHE = _build()
    return _NC_CACHE


def _prepare_in_maps(x, router_w, w1, w2, w3):
    xf = np.ascontiguousarray(np.asarray(x, np.float32).reshape(T, D))
    rw = np.asarray(router_w, np.float32)
    w1 = np.asarray(w1, np.float32)
    w2 = np.asarray(w2, np.float32)
    w3 = np.asarray(w3, np.float32)

    n13 = HP // W13_CHUNK
    w2_half = HC // 2
    w1p = np.zeros((E, D, HP), np.float32)
    w1p[:, :, :H] = w1
    w3p = np.zeros((E, D, HP), np.float32)
    w3p[:, :, :H] = w3
    w2p = np.zeros((E, HP, D), np.float32)
    w2p[:, :H, :] = w2
    w1t = np.ascontiguousarray(
        w1p.reshape(E, DC, P, n13, W13_CHUNK).transpose(0, 3, 2, 1, 4)
    )
    w3t = np.ascontiguousarray(
        w3p.reshape(E, DC, P, n13, W13_CHUNK).transpose(0, 3, 2, 1, 4)
    )
    w2t = np.ascontiguousarray(
        w2p.reshape(E, 2, w2_half, P, DC, P).transpose(0, 4, 1, 3, 2, 5)
    )

    rwt = np.ascontiguousarray(rw.T)
    tri = np.triu(np.ones((P, P), np.float32), 1)

    in_maps = []
    for c in range(NCORES):
        in_maps.append(
            {
                "x": xf[c * TC : (c + 1) * TC],
                "rwt": rwt,
                "tri": tri,
                "w1": w1t,
                "w3": w3t,
                "w2": w2t,
            }
        )
    return in_maps


def run_spmd(x, router_w, w1, w2, w3, trace=False):
    from concourse.bass_utils import run_bass_kernel_spmd

    nc = _get_nc()
    in_maps = _prepare_in_maps(x, router_w, w1, w2, w3)
    res = run_bass_kernel_spmd(nc, in_maps, list(range(NCORES)), trace=trace)
    out = np.concatenate([res.results[c]["out"] for c in range(NCORES)], axis=0)
    return out.reshape(B, N, D), res


def kernel(x, router_w, w1, w2, w3, top_k):
    assert int(top_k) == 2
    out, _ = run_spmd(x, router_w, w1, w2, w3, trace=False)
    return out
